# revision 8
# baseline (speedup 1.0000x reference)
"""Causal self-attention on 8 Trainium2 NeuronCores — v3.

Sharding: data-parallel over batch (4) x tensor-parallel over heads (2 groups
of 8). Core c handles batch c//2, head-group c%2. Each core computes
   att_out(8 heads) @ Wo[rows of its head group]  -> partial y [2048, 1024]
and the host sums the two partials per batch (the all-reduce of the hint).

v3 (vs v2 @ 326us, v1 @ 359us):
 - Host-side prep: x pre-transposed+pre-cast bf16 and pre-tiled to
   [128, 8, 2048]; weights pre-cast/pre-tiled so EVERY input lands with a
   single dma_start (issuing a DMA costs ~800ns of Sync-engine time; v2
   spent ~13us of it before the first matmul could start).
 - Attention(qc0,h0) starts right after QT/KT(m0,tq0) (~8us in; v2 waited
   52us): all other projection/V/y groups are due-scheduled PE fillers
   paced between S-pairs, assigned to phases so per-phase PE work covers
   per-phase ACT (exp) demand: proj(tq1) in qc0-phase, proj(tq2)+V(4..11)
   +y(0) in qc1, proj(tq3)+V(12..15) in qc2, y(1..11) in qc3.
 - PV lags exp by 2 key-tile pairs so it never waits on a fresh exp.
 - Merged exp: an S~T key-tile pair occupies the two banks of a
   [128,1024] PSUM tile -> ONE ACTIVATE per pair (~997ns vs 2x643ns).

Per-core pipeline (all matmuls bf16, fp32 PSUM):
  QT/KT = Wq^T x^T per m-tile [128, 2048]; V packed per-head as [128, 65]
  "V|ones" tiles (ones column yields softmax row-sums during PV).
  S^T pair [128k x 2x512q] = K_tile @ QT (+ tril mask via PE on diag
  tiles), exp on ACT (scale=1/8, no max subtraction: |S/8| < 3), O^T
  accum (V|1)^T @ expS in PSUM [65, 512]; normalize by row-64 reciprocal.
  y[tt] = (O^T)^T @ Wo_rows + bo -> DMA out fp32.
"""
import os
import numpy as np

B, T, C, H = 4, 2048, 1024, 16
D = C // H            # 64
HG = H // 2           # 8 heads per core
CG = C // 2           # 512 columns per head group
KC = C // 128         # 8 contraction tiles
NT = T // 128         # 16 row tiles
NQC = T // 512        # 4 q-chunks

_CACHE = {}
LAST_RESULT = None


def _build():
    import concourse.bacc as bacc
    import concourse.tile as tile
    from concourse import mybir

    F32 = mybir.dt.float32
    BF16 = mybir.dt.bfloat16
    AF = mybir.ActivationFunctionType

    nc = bacc.Bacc("TRN2", target_bir_lowering=False)
    # host pre-tiled layouts: partition dim first, k-tile dim second
    xt_d = nc.dram_tensor("xt", (128, KC, T), BF16, kind="ExternalInput")
    wq_d = nc.dram_tensor("wq", (128, KC, CG), BF16, kind="ExternalInput")
    wk_d = nc.dram_tensor("wk", (128, KC, CG), BF16, kind="ExternalInput")
    wv_d = nc.dram_tensor("wv", (128, KC, CG), BF16, kind="ExternalInput")
    wo_d = nc.dram_tensor("wo", (128, 4, C), BF16, kind="ExternalInput")
    bq_d = nc.dram_tensor("bq", (128, 4), F32, kind="ExternalInput")
    bk_d = nc.dram_tensor("bk", (128, 4), F32, kind="ExternalInput")
    bv_d = nc.dram_tensor("bv", (128, CG), BF16, kind="ExternalInput")
    bo_d = nc.dram_tensor("bo", (128, C), F32, kind="ExternalInput")
    mi_d = nc.dram_tensor("mi", (128, 256), BF16, kind="ExternalInput")
    y_d = nc.dram_tensor("y", (T, C), F32, kind="ExternalOutput")

    with tile.TileContext(nc) as tc:
        with tc.tile_pool(name="const", bufs=1) as cst, \
             tc.tile_pool(name="big", bufs=1) as big, \
             tc.tile_pool(name="stage", bufs=2) as stg, \
             tc.tile_pool(name="work", bufs=8) as wrk, \
             tc.tile_pool(name="ps_s", bufs=2, space="PSUM") as ps_s, \
             tc.tile_pool(name="ps_mm", bufs=2, space="PSUM") as ps_mm, \
             tc.tile_pool(name="ps_ot", bufs=2, space="PSUM") as ps_ot:

            wqs = cst.tile([128, KC, CG], BF16)
            wks = cst.tile([128, KC, CG], BF16)
            wvs = cst.tile([128, KC, CG], BF16)
            wos = cst.tile([128, 4, C], BF16)
            xts = big.tile([128, KC, T], BF16)
            # wave 0: what attention(qc0, h0) needs, in arrival order
            nc.sync.dma_start(out=wqs[:, :, 0:128], in_=wq_d[:, :, 0:128])
            nc.sync.dma_start(out=xts[:, :, 0:512], in_=xt_d[:, :, 0:512])
            nc.sync.dma_start(out=wks[:, :, 0:128], in_=wk_d[:, :, 0:128])
            nc.sync.dma_start(out=wqs[:, :, 128:CG], in_=wq_d[:, :, 128:CG])
            nc.sync.dma_start(out=wks[:, :, 128:CG], in_=wk_d[:, :, 128:CG])
            mi_sb = cst.tile([128, 256], BF16)
            nc.sync.dma_start(out=mi_sb, in_=mi_d[:, :])
            masku = mi_sb[:, 0:128]
            ident = mi_sb[:, 128:256]
            bq_sb = cst.tile([128, 4], F32)
            bk_sb = cst.tile([128, 4], F32)
            nc.sync.dma_start(out=bq_sb, in_=bq_d[:, :])
            nc.sync.dma_start(out=bk_sb, in_=bk_d[:, :])
            bvb = cst.tile([128, CG], BF16)
            nc.sync.dma_start(out=bvb, in_=bv_d[:, :])
            nc.sync.dma_start(out=wvs, in_=wv_d[:, :, :])
            # wave 1
            nc.sync.dma_start(out=xts[:, :, 512:T], in_=xt_d[:, :, 512:T])
            nc.sync.dma_start(out=wos, in_=wo_d[:, :, :])
            bob = cst.tile([128, C], F32)
            nc.sync.dma_start(out=bob, in_=bo_d[:, :])

            qt_sb = [big.tile([128, T], BF16, name=f"qt{m}") for m in range(4)]
            kt_sb = [big.tile([128, T], BF16, name=f"kt{m}") for m in range(4)]
            ot_sb = [big.tile([128, T], BF16, name=f"ot{m}") for m in range(4)]
            vones = big.tile([128, NT, HG, 65], BF16)
            nc.vector.memset(vones[:, :, :, 64:65], 1.0)

            def proj_group(tq, which, m):
                ws, bias_sb, dst = ((wqs, bq_sb, qt_sb),
                                    (wks, bk_sb, kt_sb))[which]
                pp = ps_mm.tile([128, 512], F32, name="pmm")
                for k in range(KC):
                    nc.tensor.matmul(
                        pp, lhsT=ws[:, k, 128 * m:128 * (m + 1)],
                        rhs=xts[:, k, 512 * tq:512 * (tq + 1)],
                        start=(k == 0), stop=(k == KC - 1))
                nc.vector.tensor_scalar_add(
                    dst[m][:, 512 * tq:512 * (tq + 1)], pp, bias_sb[:, m:m + 1])

            def v_group(tt):
                vp = ps_mm.tile([128, 512], F32, name="pmm")
                for k in range(KC):
                    nc.tensor.matmul(
                        vp, lhsT=xts[:, k, 128 * tt:128 * (tt + 1)],
                        rhs=wvs[:, k, :], start=(k == 0), stop=(k == KC - 1))
                nc.vector.tensor_add(
                    vones[:, tt, :, 0:64],
                    vp.rearrange("p (h d) -> p h d", h=HG),
                    bvb.rearrange("p (h d) -> p h d", h=HG))

            def y_group(tt):
                ys = stg.tile([128, C], F32, name="ysb")
                for half in range(2):
                    yp = ps_mm.tile([128, 512], F32, name="pmm")
                    for ko in range(4):
                        nc.tensor.matmul(
                            yp, lhsT=ot_sb[ko][:, 128 * tt:128 * (tt + 1)],
                            rhs=wos[:, ko, 512 * half:512 * (half + 1)],
                            start=(ko == 0), stop=(ko == 3))
                    nc.vector.tensor_add(
                        ys[:, 512 * half:512 * (half + 1)], yp,
                        bob[:, 512 * half:512 * (half + 1)])
                nc.sync.dma_start(out=y_d[128 * tt:128 * (tt + 1), :], in_=ys)

            # progressive per-ko accumulation for the last 4 y tiles: each
            # round becomes available as qc3 head-pair ko completes, so the
            # post-last-normalize tail is a single 8-matmul round.
            ys_acc = [big.tile([128, C], F32, name=f"ya{j}") for j in range(4)]

            def y_unit(tt, ko):
                ya = ys_acc[tt - 12]
                for half in range(2):
                    sl = slice(512 * half, 512 * (half + 1))
                    yp = ps_mm.tile([128, 512], F32, name="pmm")
                    nc.tensor.matmul(
                        yp, lhsT=ot_sb[ko][:, 128 * tt:128 * (tt + 1)],
                        rhs=wos[:, ko, sl], start=True, stop=True)
                    if ko == 0:
                        nc.vector.tensor_add(ya[:, sl], yp, bob[:, sl])
                    else:
                        nc.vector.tensor_add(ya[:, sl], ya[:, sl], yp)
                if ko == 3:
                    nc.sync.dma_start(out=y_d[128 * tt:128 * (tt + 1), :],
                                      in_=ya)

            def pv_pair(h, otp, item, npair):
                p, ex, qlo0, qlo1 = item
                for i, qlo in ((0, qlo0), (1, qlo1)):
                    kt = 2 * p + i
                    nc.tensor.matmul(
                        otp[:, qlo:512], lhsT=vones[:, kt, h, :],
                        rhs=ex[:, 512 * i + qlo:512 * (i + 1)],
                        start=(kt == 0), stop=(kt == 2 * npair - 1))

            def attention_head(qc, h):
                mt = h // 2
                off = 64 * (h % 2)
                npair = 2 * qc + 2
                otp = ps_ot.tile([65, 512], F32, name="potp")
                pend = []  # (pair_idx, ex_tile, qlo0, qlo1)
                for p in range(npair):
                    sp = ps_s.tile([128, 1024], F32, name="spair")
                    qlos = []
                    for i in range(2):
                        kt = 2 * p + i
                        qlo = max(0, 128 * kt - 512 * qc)
                        qlos.append(qlo)
                        diag = kt >= 4 * qc
                        nc.tensor.matmul(
                            sp[:, 512 * i + qlo:512 * (i + 1)],
                            lhsT=kt_sb[mt][off:off + 64,
                                           128 * kt:128 * (kt + 1)],
                            rhs=qt_sb[mt][off:off + 64,
                                          512 * qc + qlo:512 * (qc + 1)],
                            start=True, stop=not diag)
                        if diag:  # causal mask: S.T[k,q] += masku[q,k]
                            nc.tensor.matmul(
                                sp[:, 512 * i + qlo:512 * i + qlo + 128],
                                lhsT=masku, rhs=ident, start=False, stop=True)
                    ex = wrk.tile([128, 1024], BF16, name="exps", bufs=6)
                    if qlos[0] == qlos[1]:
                        nc.scalar.activation(
                            out=ex[:, qlos[0]:1024], in_=sp[:, qlos[0]:1024],
                            func=AF.Exp, scale=0.125)
                    else:  # diag pair: planes written from different qlo
                        nc.scalar.activation(
                            out=ex[:, qlos[0]:512], in_=sp[:, qlos[0]:512],
                            func=AF.Exp, scale=0.125)
                        nc.scalar.activation(
                            out=ex[:, 512 + qlos[1]:1024],
                            in_=sp[:, 512 + qlos[1]:1024],
                            func=AF.Exp, scale=0.125)
                    pend.append((p, ex, qlos[0], qlos[1]))
                    if len(pend) > 2:
                        pv_pair(h, otp, pend.pop(0), npair)
                    yield
                while pend:
                    pv_pair(h, otp, pend.pop(0), npair)
                # normalize: O / rowsum
                rs = wrk.tile([1, 512], F32, name="rsum", bufs=2)
                nc.vector.tensor_copy(rs, otp[64:65, :])
                rr = wrk.tile([1, 512], F32, name="rrec", bufs=2)
                nc.vector.reciprocal_approx_fast(out=rr, in_=rs)
                rb = wrk.tile([64, 512], F32, name="rbc", bufs=2)
                nc.gpsimd.partition_broadcast(rb, rr)
                nc.vector.tensor_mul(
                    ot_sb[mt][off:off + 64, 512 * qc:512 * (qc + 1)],
                    otp[0:64, :], rb)
                yield

            # ---------- schedule ----------
            def spread(lo, hi, items):
                n = len(items)
                if n == 0:
                    return []
                st = (hi - lo) / n
                return [(lo + st * (i + 1), f, a)
                        for i, (f, a) in enumerate(items)]

            P, V, Y = proj_group, v_group, y_group
            phases = {
                0: ([(2.0, V, (0,)), (2.0, V, (1,)), (2.0, V, (2,)),
                     (2.0, V, (3,)),
                     (3.0, P, (0, 0, 1)), (3.0, P, (0, 1, 1)),
                     (6.0, P, (0, 0, 2)), (6.0, P, (0, 1, 2)),
                     (9.0, P, (0, 0, 3)), (9.0, P, (0, 1, 3))]
                    + spread(10, 24, [(P, (1, 0, m)) for m in range(4)]
                             + [(P, (1, 1, m)) for m in range(4)])),
                1: ([(1.0, V, (4,)), (2.0, V, (5,)), (3.0, V, (6,)),
                     (4.0, V, (7,))]
                    + spread(6, 30, [(P, (2, w, m)) for m in range(4)
                                     for w in range(2)])
                    + spread(30, 40, [(V, (8 + j,)) for j in range(4)]
                             + [(Y, (0,))])),
                2: (spread(2, 30, [(P, (3, w, m)) for m in range(4)
                                   for w in range(2)])
                    + spread(30, 50, [(V, (12 + j,)) for j in range(4)])),
                3: spread(4, 64, [(Y, (tt,)) for tt in range(1, 12)]),
            }
            proj_group(0, 0, 0)
            proj_group(0, 1, 0)
            for qc in range(NQC):
                fillers = sorted(phases[qc], key=lambda t: t[0])
                fi, step = 0, 0
                rounds = []  # pending y_unit calls (qc3 only)
                for h in range(HG):
                    for _ in attention_head(qc, h):
                        step += 1
                        if rounds:
                            y_unit(*rounds.pop(0))
                        while fi < len(fillers) and fillers[fi][0] <= step:
                            _, f, args = fillers[fi]
                            f(*args)
                            fi += 1
                    if qc == 3 and h % 2 == 1:
                        rounds += [(tt, h // 2) for tt in range(12, 16)]
                while fi < len(fillers):
                    _, f, args = fillers[fi]
                    f(*args)
                    fi += 1
                while rounds:
                    y_unit(*rounds.pop(0))

    nc.finalize()
    return nc


def _prep(x, Wq, bq, Wk, bk, Wv, bv, Wo, bo):
    import ml_dtypes
    BF = ml_dtypes.bfloat16

    mi = np.zeros((128, 256), np.float32)
    mi[:, 0:128] = np.triu(np.full((128, 128), -1e9, np.float32), 1)
    mi[:, 128:256] = np.eye(128, dtype=np.float32)
    mi = mi.astype(BF)

    def ptile(w, nk):  # [nk*128, F] -> [128, nk, F]
        return np.ascontiguousarray(
            w.reshape(nk, 128, w.shape[1]).transpose(1, 0, 2).astype(BF))

    zero_c = np.zeros((128, C), np.float32)
    in_maps = []
    for c in range(8):
        b, g = c // 2, c % 2
        sl = slice(CG * g, CG * (g + 1))
        in_maps.append({
            "xt": ptile(x[b].T, KC),
            "wq": ptile(Wq[:, sl], KC),
            "wk": ptile(Wk[:, sl], KC),
            "wv": ptile(Wv[:, sl], KC),
            "wo": ptile(Wo[sl, :], 4),
            "bq": np.ascontiguousarray(bq[sl].reshape(4, 128).T.astype(np.float32)),
            "bk": np.ascontiguousarray(bk[sl].reshape(4, 128).T.astype(np.float32)),
            "bv": np.ascontiguousarray(
                np.broadcast_to(bv[sl].astype(BF), (128, CG))),
            "bo": np.ascontiguousarray(
                np.broadcast_to(bo.astype(np.float32), (128, C)))
            if g == 0 else zero_c,
            "mi": mi,
        })
    return in_maps


def kernel(x, Wq, bq, Wk, bk, Wv, bv, Wo, bo):
    global LAST_RESULT
    jp = os.environ.get("JAX_PLATFORMS")
    if jp is not None and "axon" not in jp:
        del os.environ["JAX_PLATFORMS"]
    from concourse.bass_utils import run_bass_kernel_spmd

    x = np.asarray(x, dtype=np.float32)
    Wq = np.asarray(Wq, dtype=np.float32)
    Wk = np.asarray(Wk, dtype=np.float32)
    Wv = np.asarray(Wv, dtype=np.float32)
    Wo = np.asarray(Wo, dtype=np.float32)
    bq = np.asarray(bq, dtype=np.float32)
    bk = np.asarray(bk, dtype=np.float32)
    bv = np.asarray(bv, dtype=np.float32)
    bo = np.asarray(bo, dtype=np.float32)

    if "nc" not in _CACHE:
        _CACHE["nc"] = _build()
    nc = _CACHE["nc"]

    in_maps = _prep(x, Wq, bq, Wk, bk, Wv, bv, Wo, bo)

    trace = bool(os.environ.get("KERNEL_TRACE"))
    try:
        res = run_bass_kernel_spmd(nc, in_maps, core_ids=list(range(8)),
                                   trace=trace)
    except Exception:
        # transient NRT exec failures (e.g. a previously wedged core) are
        # recoverable on retry
        res = run_bass_kernel_spmd(nc, in_maps, core_ids=list(range(8)),
                                   trace=trace)
    LAST_RESULT = res

    y = np.empty((B, T, C), np.float32)
    for b in range(B):
        y[b] = res.results[2 * b]["y"] + res.results[2 * b + 1]["y"]
    return y


# revision 14
# speedup vs baseline: 1.0553x; 1.0553x over previous
"""Causal self-attention on 8 Trainium2 NeuronCores — v3.

Sharding: data-parallel over batch (4) x tensor-parallel over heads (2 groups
of 8). Core c handles batch c//2, head-group c%2. Each core computes
   att_out(8 heads) @ Wo[rows of its head group]  -> partial y [2048, 1024]
and the host sums the two partials per batch (the all-reduce of the hint).

v5 (vs v4 @ 332us, v3 @ 314us, v2 @ 326us, v1 @ 359us):
 - m0 slices of Wq/Wk DMA'd first so QT/KT(m0) start ~2us earlier.
 - y(10),y(11) due-scheduled into the last head's S-pair window so the PE
   has filler work during the final normalize chain (was a 3.4us gap).
 - PV lags exp by 3 pairs in the ACT-saturated qc2/qc3 phases (2 else).

v3 (vs v2 @ 326us, v1 @ 359us):
 - Host-side prep: x pre-transposed+pre-cast bf16 and pre-tiled to
   [128, 8, 2048]; weights pre-cast/pre-tiled so EVERY input lands with a
   single dma_start (issuing a DMA costs ~800ns of Sync-engine time; v2
   spent ~13us of it before the first matmul could start).
 - Attention(qc0,h0) starts right after QT/KT(m0,tq0) (~8us in; v2 waited
   52us): all other projection/V/y groups are due-scheduled PE fillers
   paced between S-pairs, assigned to phases so per-phase PE work covers
   per-phase ACT (exp) demand: proj(tq1) in qc0-phase, proj(tq2)+V(4..11)
   +y(0) in qc1, proj(tq3)+V(12..15) in qc2, y(1..11) in qc3.
 - PV lags exp by 2 key-tile pairs so it never waits on a fresh exp.
 - Merged exp: an S~T key-tile pair occupies the two banks of a
   [128,1024] PSUM tile -> ONE ACTIVATE per pair (~997ns vs 2x643ns).

Per-core pipeline (all matmuls bf16, fp32 PSUM):
  QT/KT = Wq^T x^T per m-tile [128, 2048]; V packed per-head as [128, 65]
  "V|ones" tiles (ones column yields softmax row-sums during PV).
  S^T pair [128k x 2x512q] = K_tile @ QT (+ tril mask via PE on diag
  tiles), exp on ACT (scale=1/8, no max subtraction: |S/8| < 3), O^T
  accum (V|1)^T @ expS in PSUM [65, 512]; normalize by row-64 reciprocal.
  y[tt] = (O^T)^T @ Wo_rows + bo -> DMA out fp32.
"""
import os
import numpy as np

B, T, C, H = 4, 2048, 1024, 16
D = C // H            # 64
HG = H // 2           # 8 heads per core
CG = C // 2           # 512 columns per head group
KC = C // 128         # 8 contraction tiles
NT = T // 128         # 16 row tiles
NQC = T // 512        # 4 q-chunks

_CACHE = {}
LAST_RESULT = None


def _build():
    import concourse.bacc as bacc
    import concourse.tile as tile
    from concourse import mybir

    F32 = mybir.dt.float32
    BF16 = mybir.dt.bfloat16
    AF = mybir.ActivationFunctionType

    nc = bacc.Bacc("TRN2", target_bir_lowering=False)
    # host pre-tiled layouts: partition dim first, k-tile dim second
    xt_d = nc.dram_tensor("xt", (128, KC, T), BF16, kind="ExternalInput")
    wq_d = nc.dram_tensor("wq", (128, KC, CG), BF16, kind="ExternalInput")
    wk_d = nc.dram_tensor("wk", (128, KC, CG), BF16, kind="ExternalInput")
    wv_d = nc.dram_tensor("wv", (128, KC, CG), BF16, kind="ExternalInput")
    wo_d = nc.dram_tensor("wo", (128, 4, C), BF16, kind="ExternalInput")
    bq_d = nc.dram_tensor("bq", (128, 4), F32, kind="ExternalInput")
    bk_d = nc.dram_tensor("bk", (128, 4), F32, kind="ExternalInput")
    bv_d = nc.dram_tensor("bv", (128, CG), BF16, kind="ExternalInput")
    bo_d = nc.dram_tensor("bo", (128, C), F32, kind="ExternalInput")
    mi_d = nc.dram_tensor("mi", (128, 256), BF16, kind="ExternalInput")
    y_d = nc.dram_tensor("y", (T, C), F32, kind="ExternalOutput")

    with tile.TileContext(nc) as tc:
        with tc.tile_pool(name="const", bufs=1) as cst, \
             tc.tile_pool(name="big", bufs=1) as big, \
             tc.tile_pool(name="stage", bufs=2) as stg, \
             tc.tile_pool(name="work", bufs=8) as wrk, \
             tc.tile_pool(name="ps_s", bufs=2, space="PSUM") as ps_s, \
             tc.tile_pool(name="ps_mm", bufs=2, space="PSUM") as ps_mm, \
             tc.tile_pool(name="ps_ot", bufs=2, space="PSUM") as ps_ot:

            wqs = cst.tile([128, KC, CG], BF16)
            wks = cst.tile([128, KC, CG], BF16)
            wvs = cst.tile([128, KC, CG], BF16)
            wos = cst.tile([128, 4, C], BF16)
            xts = big.tile([128, KC, T], BF16)
            # wave 0: what attention(qc0, h0) needs, in arrival order
            nc.sync.dma_start(out=wqs[:, :, 0:128], in_=wq_d[:, :, 0:128])
            nc.sync.dma_start(out=xts[:, :, 0:512], in_=xt_d[:, :, 0:512])
            nc.sync.dma_start(out=wks[:, :, 0:128], in_=wk_d[:, :, 0:128])
            nc.sync.dma_start(out=wqs[:, :, 128:CG], in_=wq_d[:, :, 128:CG])
            nc.sync.dma_start(out=wks[:, :, 128:CG], in_=wk_d[:, :, 128:CG])
            mi_sb = cst.tile([128, 256], BF16)
            nc.sync.dma_start(out=mi_sb, in_=mi_d[:, :])
            masku = mi_sb[:, 0:128]
            ident = mi_sb[:, 128:256]
            bq_sb = cst.tile([128, 4], F32)
            bk_sb = cst.tile([128, 4], F32)
            nc.sync.dma_start(out=bq_sb, in_=bq_d[:, :])
            nc.sync.dma_start(out=bk_sb, in_=bk_d[:, :])
            bvb = cst.tile([128, CG], BF16)
            nc.sync.dma_start(out=bvb, in_=bv_d[:, :])
            nc.sync.dma_start(out=wvs, in_=wv_d[:, :, :])
            # wave 1
            nc.sync.dma_start(out=xts[:, :, 512:T], in_=xt_d[:, :, 512:T])
            nc.sync.dma_start(out=wos, in_=wo_d[:, :, :])
            bob = cst.tile([128, C], F32)
            nc.sync.dma_start(out=bob, in_=bo_d[:, :])

            qt_sb = [big.tile([128, T], BF16, name=f"qt{m}") for m in range(4)]
            kt_sb = [big.tile([128, T], BF16, name=f"kt{m}") for m in range(4)]
            ot_sb = [big.tile([128, T], BF16, name=f"ot{m}") for m in range(4)]
            vones = big.tile([128, NT, HG, 65], BF16)
            nc.vector.memset(vones[:, :, :, 64:65], 1.0)

            def proj_group(tq, which, m):
                ws, bias_sb, dst = ((wqs, bq_sb, qt_sb),
                                    (wks, bk_sb, kt_sb))[which]
                pp = ps_mm.tile([128, 512], F32, name="pmm")
                for k in range(KC):
                    nc.tensor.matmul(
                        pp, lhsT=ws[:, k, 128 * m:128 * (m + 1)],
                        rhs=xts[:, k, 512 * tq:512 * (tq + 1)],
                        start=(k == 0), stop=(k == KC - 1))
                nc.vector.tensor_scalar_add(
                    dst[m][:, 512 * tq:512 * (tq + 1)], pp, bias_sb[:, m:m + 1])

            def v_group(tt):
                vp = ps_mm.tile([128, 512], F32, name="pmm")
                for k in range(KC):
                    nc.tensor.matmul(
                        vp, lhsT=xts[:, k, 128 * tt:128 * (tt + 1)],
                        rhs=wvs[:, k, :], start=(k == 0), stop=(k == KC - 1))
                nc.vector.tensor_add(
                    vones[:, tt, :, 0:64],
                    vp.rearrange("p (h d) -> p h d", h=HG),
                    bvb.rearrange("p (h d) -> p h d", h=HG))

            def y_group(tt):
                ys = stg.tile([128, C], F32, name="ysb")
                for half in range(2):
                    yp = ps_mm.tile([128, 512], F32, name="pmm")
                    for ko in range(4):
                        nc.tensor.matmul(
                            yp, lhsT=ot_sb[ko][:, 128 * tt:128 * (tt + 1)],
                            rhs=wos[:, ko, 512 * half:512 * (half + 1)],
                            start=(ko == 0), stop=(ko == 3))
                    nc.vector.tensor_add(
                        ys[:, 512 * half:512 * (half + 1)], yp,
                        bob[:, 512 * half:512 * (half + 1)])
                nc.sync.dma_start(out=y_d[128 * tt:128 * (tt + 1), :], in_=ys)

            def pv_pair(h, otp, item, npair):
                p, ex, qlo0, qlo1 = item
                for i, qlo in ((0, qlo0), (1, qlo1)):
                    kt = 2 * p + i
                    nc.tensor.matmul(
                        otp[:, qlo:512], lhsT=vones[:, kt, h, :],
                        rhs=ex[:, 512 * i + qlo:512 * (i + 1)],
                        start=(kt == 0), stop=(kt == 2 * npair - 1))

            def attention_head(qc, h):
                mt = h // 2
                off = 64 * (h % 2)
                npair = 2 * qc + 2
                otp = ps_ot.tile([65, 512], F32, name="potp")
                pend = []  # (pair_idx, ex_tile, qlo0, qlo1)
                for p in range(npair):
                    sp = ps_s.tile([128, 1024], F32, name="spair")
                    qlos = []
                    for i in range(2):
                        kt = 2 * p + i
                        qlo = max(0, 128 * kt - 512 * qc)
                        qlos.append(qlo)
                        diag = kt >= 4 * qc
                        nc.tensor.matmul(
                            sp[:, 512 * i + qlo:512 * (i + 1)],
                            lhsT=kt_sb[mt][off:off + 64,
                                           128 * kt:128 * (kt + 1)],
                            rhs=qt_sb[mt][off:off + 64,
                                          512 * qc + qlo:512 * (qc + 1)],
                            start=True, stop=not diag)
                        if diag:  # causal mask: S.T[k,q] += masku[q,k]
                            nc.tensor.matmul(
                                sp[:, 512 * i + qlo:512 * i + qlo + 128],
                                lhsT=masku, rhs=ident, start=False, stop=True)
                    ex = wrk.tile([128, 1024], BF16, name="exps", bufs=7)
                    if qlos[0] == qlos[1]:
                        nc.scalar.activation(
                            out=ex[:, qlos[0]:1024], in_=sp[:, qlos[0]:1024],
                            func=AF.Exp, scale=0.125)
                    else:  # diag pair: planes written from different qlo
                        nc.scalar.activation(
                            out=ex[:, qlos[0]:512], in_=sp[:, qlos[0]:512],
                            func=AF.Exp, scale=0.125)
                        nc.scalar.activation(
                            out=ex[:, 512 + qlos[1]:1024],
                            in_=sp[:, 512 + qlos[1]:1024],
                            func=AF.Exp, scale=0.125)
                    pend.append((p, ex, qlos[0], qlos[1]))
                    if len(pend) > (3 if qc >= 2 else 2):
                        pv_pair(h, otp, pend.pop(0), npair)
                    yield
                while pend:
                    pv_pair(h, otp, pend.pop(0), npair)
                # normalize: O / rowsum
                rs = wrk.tile([1, 512], F32, name="rsum", bufs=2)
                nc.vector.tensor_copy(rs, otp[64:65, :])
                rr = wrk.tile([1, 512], F32, name="rrec", bufs=2)
                nc.vector.reciprocal_approx_fast(out=rr, in_=rs)
                rb = wrk.tile([64, 512], F32, name="rbc", bufs=2)
                nc.gpsimd.partition_broadcast(rb, rr)
                nc.vector.tensor_mul(
                    ot_sb[mt][off:off + 64, 512 * qc:512 * (qc + 1)],
                    otp[0:64, :], rb)
                yield

            # ---------- schedule ----------
            def spread(lo, hi, items):
                n = len(items)
                if n == 0:
                    return []
                st = (hi - lo) / n
                return [(lo + st * (i + 1), f, a)
                        for i, (f, a) in enumerate(items)]

            P, V, Y = proj_group, v_group, y_group
            phases = {
                0: ([(2.0, V, (0,)), (2.0, V, (1,)), (2.0, V, (2,)),
                     (2.0, V, (3,)),
                     (3.0, P, (0, 0, 1)), (3.0, P, (0, 1, 1)),
                     (6.0, P, (0, 0, 2)), (6.0, P, (0, 1, 2)),
                     (9.0, P, (0, 0, 3)), (9.0, P, (0, 1, 3))]
                    + spread(10, 24, [(P, (1, 0, m)) for m in range(4)]
                             + [(P, (1, 1, m)) for m in range(4)])),
                1: ([(1.0, V, (4,)), (2.0, V, (5,)), (3.0, V, (6,)),
                     (4.0, V, (7,))]
                    + spread(6, 30, [(P, (2, w, m)) for m in range(4)
                                     for w in range(2)])
                    + spread(30, 40, [(V, (8 + j,)) for j in range(4)]
                             + [(Y, (0,))])),
                2: (spread(2, 30, [(P, (3, w, m)) for m in range(4)
                                   for w in range(2)])
                    + spread(30, 50, [(V, (12 + j,)) for j in range(4)])),
                3: (spread(4, 58, [(Y, (tt,)) for tt in range(1, 10)])
                    + [(70.5, Y, (10,)), (71.5, Y, (11,))]),
            }
            proj_group(0, 0, 0)
            proj_group(0, 1, 0)
            for qc in range(NQC):
                fillers = sorted(phases[qc], key=lambda t: t[0])
                fi, step = 0, 0
                for h in range(HG):
                    for _ in attention_head(qc, h):
                        step += 1
                        while fi < len(fillers) and fillers[fi][0] <= step:
                            _, f, args = fillers[fi]
                            f(*args)
                            fi += 1
                while fi < len(fillers):
                    _, f, args = fillers[fi]
                    f(*args)
                    fi += 1
            for tt in range(12, 16):
                y_group(tt)

    nc.finalize()
    return nc


def _prep(x, Wq, bq, Wk, bk, Wv, bv, Wo, bo):
    import ml_dtypes
    BF = ml_dtypes.bfloat16

    mi = np.zeros((128, 256), np.float32)
    mi[:, 0:128] = np.triu(np.full((128, 128), -1e9, np.float32), 1)
    mi[:, 128:256] = np.eye(128, dtype=np.float32)
    mi = mi.astype(BF)

    def ptile(w, nk):  # [nk*128, F] -> [128, nk, F]
        return np.ascontiguousarray(
            w.reshape(nk, 128, w.shape[1]).transpose(1, 0, 2).astype(BF))

    zero_c = np.zeros((128, C), np.float32)
    in_maps = []
    for c in range(8):
        b, g = c // 2, c % 2
        sl = slice(CG * g, CG * (g + 1))
        in_maps.append({
            "xt": ptile(x[b].T, KC),
            "wq": ptile(Wq[:, sl], KC),
            "wk": ptile(Wk[:, sl], KC),
            "wv": ptile(Wv[:, sl], KC),
            "wo": ptile(Wo[sl, :], 4),
            "bq": np.ascontiguousarray(bq[sl].reshape(4, 128).T.astype(np.float32)),
            "bk": np.ascontiguousarray(bk[sl].reshape(4, 128).T.astype(np.float32)),
            "bv": np.ascontiguousarray(
                np.broadcast_to(bv[sl].astype(BF), (128, CG))),
            "bo": np.ascontiguousarray(
                np.broadcast_to(bo.astype(np.float32), (128, C)))
            if g == 0 else zero_c,
            "mi": mi,
        })
    return in_maps


def kernel(x, Wq, bq, Wk, bk, Wv, bv, Wo, bo):
    global LAST_RESULT
    jp = os.environ.get("JAX_PLATFORMS")
    if jp is not None and "axon" not in jp:
        del os.environ["JAX_PLATFORMS"]
    from concourse.bass_utils import run_bass_kernel_spmd

    x = np.asarray(x, dtype=np.float32)
    Wq = np.asarray(Wq, dtype=np.float32)
    Wk = np.asarray(Wk, dtype=np.float32)
    Wv = np.asarray(Wv, dtype=np.float32)
    Wo = np.asarray(Wo, dtype=np.float32)
    bq = np.asarray(bq, dtype=np.float32)
    bk = np.asarray(bk, dtype=np.float32)
    bv = np.asarray(bv, dtype=np.float32)
    bo = np.asarray(bo, dtype=np.float32)

    if "nc" not in _CACHE:
        _CACHE["nc"] = _build()
    nc = _CACHE["nc"]

    in_maps = _prep(x, Wq, bq, Wk, bk, Wv, bv, Wo, bo)

    trace = bool(os.environ.get("KERNEL_TRACE"))
    try:
        res = run_bass_kernel_spmd(nc, in_maps, core_ids=list(range(8)),
                                   trace=trace)
    except Exception:
        # transient NRT exec failures (e.g. a previously wedged core) are
        # recoverable on retry
        res = run_bass_kernel_spmd(nc, in_maps, core_ids=list(range(8)),
                                   trace=trace)
    LAST_RESULT = res

    y = np.empty((B, T, C), np.float32)
    for b in range(B):
        y[b] = res.results[2 * b]["y"] + res.results[2 * b + 1]["y"]
    return y


# revision 15
# speedup vs baseline: 1.1078x; 1.0497x over previous
"""Causal self-attention on 8 Trainium2 NeuronCores — v3.

Sharding: data-parallel over batch (4) x tensor-parallel over heads (2 groups
of 8). Core c handles batch c//2, head-group c%2. Each core computes
   att_out(8 heads) @ Wo[rows of its head group]  -> partial y [2048, 1024]
and the host sums the two partials per batch (the all-reduce of the hint).

v5 (vs v4 @ 332us, v3 @ 314us, v2 @ 326us, v1 @ 359us):
 - m0 slices of Wq/Wk DMA'd first so QT/KT(m0) start ~2us earlier.
 - y(10),y(11) due-scheduled into the last head's S-pair window so the PE
   has filler work during the final normalize chain (was a 3.4us gap).
 - PV lags exp by 3 pairs in the ACT-saturated qc2/qc3 phases (2 else).

v3 (vs v2 @ 326us, v1 @ 359us):
 - Host-side prep: x pre-transposed+pre-cast bf16 and pre-tiled to
   [128, 8, 2048]; weights pre-cast/pre-tiled so EVERY input lands with a
   single dma_start (issuing a DMA costs ~800ns of Sync-engine time; v2
   spent ~13us of it before the first matmul could start).
 - Attention(qc0,h0) starts right after QT/KT(m0,tq0) (~8us in; v2 waited
   52us): all other projection/V/y groups are due-scheduled PE fillers
   paced between S-pairs, assigned to phases so per-phase PE work covers
   per-phase ACT (exp) demand: proj(tq1) in qc0-phase, proj(tq2)+V(4..11)
   +y(0) in qc1, proj(tq3)+V(12..15) in qc2, y(1..11) in qc3.
 - PV lags exp by 2 key-tile pairs so it never waits on a fresh exp.
 - Merged exp: an S~T key-tile pair occupies the two banks of a
   [128,1024] PSUM tile -> ONE ACTIVATE per pair (~997ns vs 2x643ns).

Per-core pipeline (all matmuls bf16, fp32 PSUM):
  QT/KT = Wq^T x^T per m-tile [128, 2048]; V packed per-head as [128, 65]
  "V|ones" tiles (ones column yields softmax row-sums during PV).
  S^T pair [128k x 2x512q] = K_tile @ QT (+ tril mask via PE on diag
  tiles), exp on ACT (scale=1/8, no max subtraction: |S/8| < 3), O^T
  accum (V|1)^T @ expS in PSUM [65, 512]; normalize by row-64 reciprocal.
  y[tt] = (O^T)^T @ Wo_rows + bo -> DMA out fp32.
"""
import os
import numpy as np

B, T, C, H = 4, 2048, 1024, 16
D = C // H            # 64
HG = H // 2           # 8 heads per core
CG = C // 2           # 512 columns per head group
KC = C // 128         # 8 contraction tiles
NT = T // 128         # 16 row tiles
NQC = T // 512        # 4 q-chunks

_CACHE = {}
LAST_RESULT = None


def _build():
    import concourse.bacc as bacc
    import concourse.tile as tile
    from concourse import mybir

    F32 = mybir.dt.float32
    BF16 = mybir.dt.bfloat16
    AF = mybir.ActivationFunctionType

    nc = bacc.Bacc("TRN2", target_bir_lowering=False)
    # host pre-tiled layouts: partition dim first, k-tile dim second
    xt_d = nc.dram_tensor("xt", (128, KC, T), BF16, kind="ExternalInput")
    wq_d = nc.dram_tensor("wq", (128, KC, CG), BF16, kind="ExternalInput")
    wk_d = nc.dram_tensor("wk", (128, KC, CG), BF16, kind="ExternalInput")
    wv_d = nc.dram_tensor("wv", (128, KC, CG), BF16, kind="ExternalInput")
    wo_d = nc.dram_tensor("wo", (128, 4, C), BF16, kind="ExternalInput")
    bq_d = nc.dram_tensor("bq", (128, 4), F32, kind="ExternalInput")
    bk_d = nc.dram_tensor("bk", (128, 4), F32, kind="ExternalInput")
    bv_d = nc.dram_tensor("bv", (128, CG), BF16, kind="ExternalInput")
    bo_d = nc.dram_tensor("bo", (128, C), F32, kind="ExternalInput")
    mi_d = nc.dram_tensor("mi", (128, 256), BF16, kind="ExternalInput")
    y_d = nc.dram_tensor("y", (T, C), F32, kind="ExternalOutput")

    with tile.TileContext(nc) as tc:
        with tc.tile_pool(name="const", bufs=1) as cst, \
             tc.tile_pool(name="big", bufs=1) as big, \
             tc.tile_pool(name="stage", bufs=2) as stg, \
             tc.tile_pool(name="work", bufs=8) as wrk, \
             tc.tile_pool(name="ps_s", bufs=2, space="PSUM") as ps_s, \
             tc.tile_pool(name="ps_mm", bufs=2, space="PSUM") as ps_mm, \
             tc.tile_pool(name="ps_ot", bufs=2, space="PSUM") as ps_ot:

            wqs = cst.tile([128, KC, CG], BF16)
            wks = cst.tile([128, KC, CG], BF16)
            wvs = cst.tile([128, KC, CG], BF16)
            wos = cst.tile([128, 4, C], BF16)
            xts = big.tile([128, KC, T], BF16)
            # wave 0: what attention(qc0, h0) needs, in arrival order
            nc.sync.dma_start(out=wqs[:, :, 0:128], in_=wq_d[:, :, 0:128])
            nc.sync.dma_start(out=xts[:, :, 0:512], in_=xt_d[:, :, 0:512])
            nc.sync.dma_start(out=wks[:, :, 0:128], in_=wk_d[:, :, 0:128])
            nc.sync.dma_start(out=wqs[:, :, 128:CG], in_=wq_d[:, :, 128:CG])
            nc.sync.dma_start(out=wks[:, :, 128:CG], in_=wk_d[:, :, 128:CG])
            mi_sb = cst.tile([128, 256], BF16)
            nc.sync.dma_start(out=mi_sb, in_=mi_d[:, :])
            masku = mi_sb[:, 0:128]
            ident = mi_sb[:, 128:256]
            bq_sb = cst.tile([128, 4], F32)
            bk_sb = cst.tile([128, 4], F32)
            nc.sync.dma_start(out=bq_sb, in_=bq_d[:, :])
            nc.sync.dma_start(out=bk_sb, in_=bk_d[:, :])
            bvb = cst.tile([128, CG], BF16)
            nc.sync.dma_start(out=bvb, in_=bv_d[:, :])
            nc.sync.dma_start(out=wvs, in_=wv_d[:, :, :])
            # wave 1
            nc.sync.dma_start(out=xts[:, :, 512:T], in_=xt_d[:, :, 512:T])
            nc.sync.dma_start(out=wos, in_=wo_d[:, :, :])
            bob = cst.tile([128, C], F32)
            nc.sync.dma_start(out=bob, in_=bo_d[:, :])

            qt_sb = [big.tile([128, T], BF16, name=f"qt{m}") for m in range(4)]
            kt_sb = [big.tile([128, T], BF16, name=f"kt{m}") for m in range(4)]
            ot_sb = [big.tile([128, T], BF16, name=f"ot{m}") for m in range(4)]
            vones = big.tile([128, NT, HG, 65], BF16)
            nc.vector.memset(vones[:, :, :, 64:65], 1.0)

            def proj_group(tq, which, m):
                ws, bias_sb, dst = ((wqs, bq_sb, qt_sb),
                                    (wks, bk_sb, kt_sb))[which]
                pp = ps_mm.tile([128, 512], F32, name="pmm")
                for k in range(KC):
                    nc.tensor.matmul(
                        pp, lhsT=ws[:, k, 128 * m:128 * (m + 1)],
                        rhs=xts[:, k, 512 * tq:512 * (tq + 1)],
                        start=(k == 0), stop=(k == KC - 1))
                nc.vector.tensor_scalar_add(
                    dst[m][:, 512 * tq:512 * (tq + 1)], pp, bias_sb[:, m:m + 1])

            def v_group(tt):
                vp = ps_mm.tile([128, 512], F32, name="pmm")
                for k in range(KC):
                    nc.tensor.matmul(
                        vp, lhsT=xts[:, k, 128 * tt:128 * (tt + 1)],
                        rhs=wvs[:, k, :], start=(k == 0), stop=(k == KC - 1))
                nc.vector.tensor_add(
                    vones[:, tt, :, 0:64],
                    vp.rearrange("p (h d) -> p h d", h=HG),
                    bvb.rearrange("p (h d) -> p h d", h=HG))

            def y_group(tt):
                ys = stg.tile([128, C], F32, name="ysb")
                for half in range(2):
                    yp = ps_mm.tile([128, 512], F32, name="pmm")
                    for ko in range(4):
                        nc.tensor.matmul(
                            yp, lhsT=ot_sb[ko][:, 128 * tt:128 * (tt + 1)],
                            rhs=wos[:, ko, 512 * half:512 * (half + 1)],
                            start=(ko == 0), stop=(ko == 3))
                    nc.vector.tensor_add(
                        ys[:, 512 * half:512 * (half + 1)], yp,
                        bob[:, 512 * half:512 * (half + 1)])
                nc.sync.dma_start(out=y_d[128 * tt:128 * (tt + 1), :], in_=ys)

            def pv_pair(h, otp, item, npair):
                p, ex, qlo0, qlo1 = item
                for i, qlo in ((0, qlo0), (1, qlo1)):
                    kt = 2 * p + i
                    nc.tensor.matmul(
                        otp[:, qlo:512], lhsT=vones[:, kt, h, :],
                        rhs=ex[:, 512 * i + qlo:512 * (i + 1)],
                        start=(kt == 0), stop=(kt == 2 * npair - 1))

            def attention_head(qc, h):
                mt = h // 2
                off = 64 * (h % 2)
                npair = 2 * qc + 2
                otp = ps_ot.tile([65, 512], F32, name="potp")
                pend = []  # (pair_idx, ex_tile, qlo0, qlo1)
                for p in range(npair):
                    sp = ps_s.tile([128, 1024], F32, name="spair")
                    qlos = []
                    for i in range(2):
                        kt = 2 * p + i
                        qlo = max(0, 128 * kt - 512 * qc)
                        qlos.append(qlo)
                        nc.tensor.matmul(
                            sp[:, 512 * i + qlo:512 * (i + 1)],
                            lhsT=kt_sb[mt][off:off + 64,
                                           128 * kt:128 * (kt + 1)],
                            rhs=qt_sb[mt][off:off + 64,
                                          512 * qc + qlo:512 * (qc + 1)],
                            start=True, stop=True)
                    ex = wrk.tile([128, 1024], BF16, name="exps", bufs=7)
                    if qlos[0] == qlos[1]:
                        nc.scalar.activation(
                            out=ex[:, qlos[0]:1024], in_=sp[:, qlos[0]:1024],
                            func=AF.Exp, scale=0.125)
                    else:  # diag pair: planes written from different qlo
                        nc.scalar.activation(
                            out=ex[:, qlos[0]:512], in_=sp[:, qlos[0]:512],
                            func=AF.Exp, scale=0.125)
                        nc.scalar.activation(
                            out=ex[:, 512 + qlos[1]:1024],
                            in_=sp[:, 512 + qlos[1]:1024],
                            func=AF.Exp, scale=0.125)
                    for i in range(2):
                        kt = 2 * p + i
                        if kt >= 4 * qc:  # causal: zero exp where q < k
                            qlo = qlos[i]
                            nc.gpsimd.affine_select(
                                out=ex[:, 512 * i + qlo:512 * i + qlo + 128],
                                in_=ex[:, 512 * i + qlo:512 * i + qlo + 128],
                                compare_op=mybir.AluOpType.is_ge,
                                fill=0.0, base=0, pattern=[[1, 128]],
                                channel_multiplier=-1)
                    pend.append((p, ex, qlos[0], qlos[1]))
                    if len(pend) > (3 if qc >= 2 else 2):
                        pv_pair(h, otp, pend.pop(0), npair)
                    yield
                while pend:
                    pv_pair(h, otp, pend.pop(0), npair)
                # normalize: O / rowsum
                rs = wrk.tile([1, 512], F32, name="rsum", bufs=2)
                nc.vector.tensor_copy(rs, otp[64:65, :])
                rr = wrk.tile([1, 512], F32, name="rrec", bufs=2)
                nc.vector.reciprocal_approx_fast(out=rr, in_=rs)
                rb = wrk.tile([64, 512], F32, name="rbc", bufs=2)
                nc.gpsimd.partition_broadcast(rb, rr)
                nc.vector.tensor_mul(
                    ot_sb[mt][off:off + 64, 512 * qc:512 * (qc + 1)],
                    otp[0:64, :], rb)
                yield

            # ---------- schedule ----------
            def spread(lo, hi, items):
                n = len(items)
                if n == 0:
                    return []
                st = (hi - lo) / n
                return [(lo + st * (i + 1), f, a)
                        for i, (f, a) in enumerate(items)]

            P, V, Y = proj_group, v_group, y_group
            phases = {
                0: ([(2.0, V, (0,)), (2.0, V, (1,)), (2.0, V, (2,)),
                     (2.0, V, (3,)),
                     (3.0, P, (0, 0, 1)), (3.0, P, (0, 1, 1)),
                     (6.0, P, (0, 0, 2)), (6.0, P, (0, 1, 2)),
                     (9.0, P, (0, 0, 3)), (9.0, P, (0, 1, 3))]
                    + spread(10, 24, [(P, (1, 0, m)) for m in range(4)]
                             + [(P, (1, 1, m)) for m in range(4)])),
                1: ([(1.0, V, (4,)), (2.0, V, (5,)), (3.0, V, (6,)),
                     (4.0, V, (7,))]
                    + spread(6, 30, [(P, (2, w, m)) for m in range(4)
                                     for w in range(2)])
                    + spread(30, 40, [(V, (8 + j,)) for j in range(4)]
                             + [(Y, (0,))])),
                2: (spread(2, 30, [(P, (3, w, m)) for m in range(4)
                                   for w in range(2)])
                    + spread(30, 50, [(V, (12 + j,)) for j in range(4)])),
                3: (spread(4, 50, [(Y, (tt,)) for tt in range(1, 8)])
                    + [(70.2, Y, (8,)), (70.7, Y, (9,)),
                       (71.3, Y, (10,)), (71.8, Y, (11,))]),
            }
            proj_group(0, 0, 0)
            proj_group(0, 1, 0)
            for qc in range(NQC):
                fillers = sorted(phases[qc], key=lambda t: t[0])
                fi, step = 0, 0
                for h in range(HG):
                    for _ in attention_head(qc, h):
                        step += 1
                        while fi < len(fillers) and fillers[fi][0] <= step:
                            _, f, args = fillers[fi]
                            f(*args)
                            fi += 1
                while fi < len(fillers):
                    _, f, args = fillers[fi]
                    f(*args)
                    fi += 1
            for tt in range(12, 16):
                y_group(tt)

    nc.finalize()
    return nc


def _prep(x, Wq, bq, Wk, bk, Wv, bv, Wo, bo):
    import ml_dtypes
    BF = ml_dtypes.bfloat16

    mi = np.zeros((128, 256), np.float32)
    mi[:, 0:128] = np.triu(np.full((128, 128), -1e9, np.float32), 1)
    mi[:, 128:256] = np.eye(128, dtype=np.float32)
    mi = mi.astype(BF)

    def ptile(w, nk):  # [nk*128, F] -> [128, nk, F]
        return np.ascontiguousarray(
            w.reshape(nk, 128, w.shape[1]).transpose(1, 0, 2).astype(BF))

    zero_c = np.zeros((128, C), np.float32)
    in_maps = []
    for c in range(8):
        b, g = c // 2, c % 2
        sl = slice(CG * g, CG * (g + 1))
        in_maps.append({
            "xt": ptile(x[b].T, KC),
            "wq": ptile(Wq[:, sl], KC),
            "wk": ptile(Wk[:, sl], KC),
            "wv": ptile(Wv[:, sl], KC),
            "wo": ptile(Wo[sl, :], 4),
            "bq": np.ascontiguousarray(bq[sl].reshape(4, 128).T.astype(np.float32)),
            "bk": np.ascontiguousarray(bk[sl].reshape(4, 128).T.astype(np.float32)),
            "bv": np.ascontiguousarray(
                np.broadcast_to(bv[sl].astype(BF), (128, CG))),
            "bo": np.ascontiguousarray(
                np.broadcast_to(bo.astype(np.float32), (128, C)))
            if g == 0 else zero_c,
            "mi": mi,
        })
    return in_maps


def kernel(x, Wq, bq, Wk, bk, Wv, bv, Wo, bo):
    global LAST_RESULT
    jp = os.environ.get("JAX_PLATFORMS")
    if jp is not None and "axon" not in jp:
        del os.environ["JAX_PLATFORMS"]
    from concourse.bass_utils import run_bass_kernel_spmd

    x = np.asarray(x, dtype=np.float32)
    Wq = np.asarray(Wq, dtype=np.float32)
    Wk = np.asarray(Wk, dtype=np.float32)
    Wv = np.asarray(Wv, dtype=np.float32)
    Wo = np.asarray(Wo, dtype=np.float32)
    bq = np.asarray(bq, dtype=np.float32)
    bk = np.asarray(bk, dtype=np.float32)
    bv = np.asarray(bv, dtype=np.float32)
    bo = np.asarray(bo, dtype=np.float32)

    if "nc" not in _CACHE:
        _CACHE["nc"] = _build()
    nc = _CACHE["nc"]

    in_maps = _prep(x, Wq, bq, Wk, bk, Wv, bv, Wo, bo)

    trace = bool(os.environ.get("KERNEL_TRACE"))
    try:
        res = run_bass_kernel_spmd(nc, in_maps, core_ids=list(range(8)),
                                   trace=trace)
    except Exception:
        # transient NRT exec failures (e.g. a previously wedged core) are
        # recoverable on retry
        res = run_bass_kernel_spmd(nc, in_maps, core_ids=list(range(8)),
                                   trace=trace)
    LAST_RESULT = res

    y = np.empty((B, T, C), np.float32)
    for b in range(B):
        y[b] = res.results[2 * b]["y"] + res.results[2 * b + 1]["y"]
    return y
